# revision 19
# baseline (speedup 1.0000x reference)
"""BasicNCA (neural cellular automaton) Trainium2 kernel, 8-core SPMD.

Reference computation (per step, 32 steps):
  p  = depthwise3x3(s, [identity, sobel_x, sobel_y])   # (B, 3C, H, W)
  h  = relu(w1 @ p + b1)                               # (B, 64, H, W)
  d  = w2 @ h + b2                                     # (B, C, H, W)
  s += d * (mask < 0.5)

Implementation notes:
- The perception conv + first 1x1 conv fuse into one effective 3x3 conv with
  weights Weff[o, c, dy, dx]; computed on the PE as 9 shifted "tap" matmuls
  accumulating in PSUM (fp32r operands, full rate at free dim >= 256).
- Sharding: core i handles batch i//2, H-half i%2, with a 32-row taper of
  redundant compute instead of per-step halo exchange between cores
  (validity shrinks 1 row/step; 32 steps consume exactly the margin).
- A core's 96-row slab splits into 4 sub-slabs of 24 rows on the 4 SBUF
  partition quadrants (channels 0-23 of each), so DVE/ACT elementwise work
  runs ~120 lanes wide.
- This walrus build rejects column tile positions != 0 and crashes on PSUM
  accumulation chains that change tile_position mid-chain. So: taps process
  slab PAIRS with K=56 block lhsT (both slabs' h stacked in the output's
  partition halves, one tile_position per chain); the second 1x1 conv runs
  in full 128x128 mode (K=128 over both slabs' h, M=120 with per-slab
  24-column blocks so deltas land on the right partition quadrants).
- Masks are pre-replicated across channel lanes on the host and streamed per
  step (DMA is otherwise idle).
"""

import sys

sys.path.insert(0, "/opt/trn_rl_repo")

import numpy as np

import concourse.bass as bass
import concourse.bacc as bacc
import concourse.tile as tile
import concourse.mybir as mybir

dt = mybir.dt

B, C, H, W = 4, 24, 128, 128
HID = 64
FIRE_RATE = 0.5
N_CORES = 8

SH = 96            # slab rows per core (64 own + 32 taper)
SR = 24            # rows per sub-slab (one partition quadrant)
FW = W + 2         # padded row width (130)
FR = SR + 2        # frame rows per sub-slab (26)
FRAME_OFF = 4      # leading guard elems so tap offset -1 stays in-bounds
FRAME = FR * FW    # 3380
S_FREE = FRAME_OFF + FRAME + 4
COMP = SR * FW     # 3120 compact free size (real rows 0..23)
NCH = 390          # chunk = 3 rows
NCHUNK = COMP // NCH  # 8

LAST_EXEC_NS = None
_cache = {}


def _taps():
    # correlation taps: out(y, x) = sum_{dy,dx} in(y+dy-1, x+dx-1) * k[dy, dx]
    return [(dy, dx) for dy in range(3) for dx in range(3)]


def _build_program(steps, apply_b2):
    nc = bacc.Bacc("TRN2", target_bir_lowering=False, debug=False,
                   num_devices=N_CORES)

    s_d = nc.dram_tensor("s0", [128, S_FREE], dt.float32r, kind="ExternalInput")
    m_d = nc.dram_tensor("masks", [steps, 128, COMP], dt.float32,
                         kind="ExternalInput")
    tapw_d = nc.dram_tensor("tapw", [128, 9 * 128], dt.float32r,
                            kind="ExternalInput")
    w2b_d = nc.dram_tensor("w2b", [128, 2 * 120], dt.float32r,
                           kind="ExternalInput")
    b2r_d = nc.dram_tensor("b2r", [128, 1], dt.float32, kind="ExternalInput")
    b1_d = nc.dram_tensor("b1v", [128, 1], dt.float32, kind="ExternalInput")
    out_d = nc.dram_tensor("out", [128, SR * W], dt.float32,
                           kind="ExternalOutput")

    with tile.TileContext(nc) as tc:
        with tc.tile_pool(name="persist", bufs=1) as pp, \
             tc.tile_pool(name="mpool", bufs=2) as mpool, \
             tc.tile_pool(name="hsb", bufs=3) as hsbp, \
             tc.tile_pool(name="upool", bufs=2) as upool, \
             tc.tile_pool(name="hps", bufs=3, space="PSUM") as hps_pool, \
             tc.tile_pool(name="dps", bufs=1, space="PSUM") as dps_pool:

            s_sb = pp.tile([128, S_FREE], dt.float32r)
            tapw = pp.tile([128, 9 * 128], dt.float32r)
            w2b = pp.tile([128, 2 * 120], dt.float32r)
            b2r = pp.tile([128, 1], dt.float32)
            b1v = pp.tile([128, 1], dt.float32)

            nc.sync.dma_start(s_sb[:], s_d[:])
            nc.sync.dma_start(tapw[:], tapw_d[:])
            nc.sync.dma_start(w2b[:], w2b_d[:])
            nc.sync.dma_start(b2r[:], b2r_d[:])
            nc.sync.dma_start(b1v[:], b1_d[:])

            taps = _taps()

            for t in range(steps):
                m_sb = mpool.tile([128, COMP], dt.float32, tag="m")
                nc.sync.dma_start(m_sb[:], m_d[t])

                u_sb = upool.tile([128, COMP], dt.float32, tag="u")

                for k in range(NCHUNK // 2):        # chunk pairs
                    hsb_list = []
                    for p in range(2):              # slab pairs {0,1} / {2,3}
                        base = 64 * p
                        hps = hps_pool.tile([128, 1024], dt.float32,
                                            tag="hps")
                        for cc in range(2):
                            c = 2 * k + cc
                            for ti, (dy, dx) in enumerate(taps):
                                off = (FRAME_OFF + (3 * c + dy) * FW
                                       + dx - 1)
                                nc.tensor.matmul(
                                    hps[:, 512 * cc:512 * cc + NCH],
                                    tapw[base:base + 56,
                                         128 * ti:128 * ti + 128],
                                    s_sb[base:base + 56, off:off + NCH],
                                    start=(ti == 0), stop=(ti == 8),
                                    tile_position=(base, 0),
                                )
                        # relu + b1 for both chunks of this slab pair
                        hsb = hsbp.tile([128, 2 * NCH], dt.float32r,
                                        tag=f"hsb{p}")
                        hsb_list.append(hsb)
                        nc.scalar.activation(
                            hsb[:].rearrange("p (b x) -> p b x", x=NCH),
                            hps[:].rearrange("p (b x) -> p b x", b=2)[:, :, 0:NCH],
                            mybir.ActivationFunctionType.Relu,
                            bias=b1v[:, 0:1],
                        )

                    dps = dps_pool.tile([128, 1024], dt.float32, tag="dps")
                    for cc in range(2):
                        for p in range(2):
                            nc.tensor.matmul(
                                dps[0:120, 512 * cc:512 * cc + NCH],
                                w2b[:, 120 * p:120 * p + 120],
                                hsb_list[p][:, NCH * cc:NCH * cc + NCH],
                                start=(p == 0), stop=(p == 1),
                            )
                    if apply_b2:
                        # delta += b2 (per-partition scalar), in psum
                        nc.vector.tensor_scalar_add(
                            dps[0:120].rearrange(
                                "p (b x) -> p b x", b=2)[:, :, 0:NCH],
                            dps[0:120].rearrange(
                                "p (b x) -> p b x", b=2)[:, :, 0:NCH],
                            b2r[0:120, 0:1],
                        )

                    # u = (m < 0.5) * delta for this chunk pair
                    nc.vector.scalar_tensor_tensor(
                        u_sb[0:120, 780 * k:780 * k + 780].rearrange(
                            "p (b x) -> p b x", x=NCH),
                        m_sb[0:120, 780 * k:780 * k + 780].rearrange(
                            "p (b x) -> p b x", x=NCH),
                        FIRE_RATE,
                        dps[0:120].rearrange("p (b x) -> p b x", b=2)[:, :, 0:NCH],
                        mybir.AluOpType.is_lt,
                        mybir.AluOpType.mult,
                    )

                # s += u over the real area (frame rows 1..24)
                nc.vector.tensor_add(
                    s_sb[0:120, FRAME_OFF + FW:FRAME_OFF + FW + COMP],
                    s_sb[0:120, FRAME_OFF + FW:FRAME_OFF + FW + COMP],
                    u_sb[0:120],
                )

                # intra-core halo refresh between sub-slabs
                if t < steps - 1:
                    for g in range(3):
                        nc.sync.dma_start(
                            s_sb[32 * g:32 * g + 24,
                                 FRAME_OFF + 25 * FW:FRAME_OFF + 25 * FW + FW],
                            s_sb[32 * (g + 1):32 * (g + 1) + 24,
                                 FRAME_OFF + FW:FRAME_OFF + FW + FW],
                        )
                        nc.sync.dma_start(
                            s_sb[32 * (g + 1):32 * (g + 1) + 24,
                                 FRAME_OFF:FRAME_OFF + FW],
                            s_sb[32 * g:32 * g + 24,
                                 FRAME_OFF + 24 * FW:FRAME_OFF + 24 * FW + FW],
                        )

            # write back real pixels (frame rows 1..24, cols 1..128)
            a0 = FRAME_OFF + FW + 1
            nc.sync.dma_start(
                out_d[:].rearrange("p (r x) -> p r x", x=W),
                s_sb[:, a0:a0 + SR * FW].rearrange(
                    "p (r x) -> p r x", x=FW)[:, :, 0:W].bitcast(dt.float32),
            )

    nc.compile()
    return nc


def _prep_weights(w1, b1, w2, b2):
    sx = np.array([[-1, 0, 1], [-2, 0, 2], [-1, 0, 1]], np.float32) / 8.0
    sy = sx.T.copy()
    ident = np.zeros((3, 3), np.float32)
    ident[1, 1] = 1.0
    # Weff[o, c, dy, dx]
    weff = (np.einsum("oc,yx->ocyx", w1[:, 0::3], ident)
            + np.einsum("oc,yx->ocyx", w1[:, 1::3], sx)
            + np.einsum("oc,yx->ocyx", w1[:, 2::3], sy)).astype(np.float32)

    # pair-tap lhsT: K=56 rows (quadrants q, q+1 channels), M=128
    # rows 0-23 -> h of even slab at out partitions 0-63,
    # rows 32-55 -> h of odd slab at out partitions 64-127.
    tapw = np.zeros((128, 9 * 128), np.float32)
    for ti, (dy, dx) in enumerate(_taps()):
        wt = weff[:, :, dy, dx].T          # [24, 64]
        for p in range(2):
            base = 64 * p
            tapw[base:base + 24, 128 * ti:128 * ti + 64] = wt
            tapw[base + 32:base + 56, 128 * ti + 64:128 * ti + 128] = wt

    # layer2 lhsT per pair: K=128 (both h halves), M=120 with 24-col blocks
    # placing each slab's delta on its partition quadrant.
    w2b = np.zeros((128, 2 * 120), np.float32)
    for p in range(2):
        ge, go = 2 * p, 2 * p + 1
        w2b[0:64, 120 * p + 32 * ge:120 * p + 32 * ge + 24] = w2.T
        w2b[64:128, 120 * p + 32 * go:120 * p + 32 * go + 24] = w2.T

    b2r = np.zeros((128, 1), np.float32)
    b1v = np.zeros((128, 1), np.float32)
    for g in range(4):
        b2r[32 * g:32 * g + 24, 0] = b2
    b1v[0:64, 0] = b1
    b1v[64:128, 0] = b1
    return tapw, w2b, b2r, b1v


def _prep_state(state):
    """state (B, C, H, W) -> per-core [128, S_FREE] framed slabs."""
    bufs = []
    for core in range(N_CORES):
        b = core // 2
        top = (core % 2) == 0
        r0 = 0 if top else H - SH
        buf = np.zeros((128, S_FREE), np.float32)
        for ch in range(C):
            full = np.zeros((SH + 2, FW), np.float32)
            full[1:SH + 1, 1:W + 1] = state[b, ch, r0:r0 + SH, :]
            if r0 > 0:
                full[0, 1:W + 1] = state[b, ch, r0 - 1, :]
            if r0 + SH < H:
                full[SH + 1, 1:W + 1] = state[b, ch, r0 + SH, :]
            for g in range(4):
                fr = full[g * SR:g * SR + FR, :]
                buf[32 * g + ch, FRAME_OFF:FRAME_OFF + FRAME] = fr.reshape(-1)
        bufs.append(buf)
    return bufs


def _prep_masks(masks):
    """masks (S, B, 1, H, W) -> per-core [S, 128, COMP] fire-padded."""
    S = masks.shape[0]
    bufs = []
    for core in range(N_CORES):
        b = core // 2
        top = (core % 2) == 0
        r0 = 0 if top else H - SH
        mb = np.ones((S, 128, COMP), np.float32)
        mrows = np.ones((S, SH, FW), np.float32)
        mrows[:, :, 1:W + 1] = masks[:, b, 0, r0:r0 + SH, :]
        for g in range(4):
            seg = mrows[:, g * SR:(g + 1) * SR, :].reshape(S, COMP)
            mb[:, 32 * g:32 * g + C, :] = seg[:, None, :]
        bufs.append(mb)
    return bufs


def kernel(state, w1, b1, w2, b2, masks):
    state = np.asarray(state)
    w1, b1 = np.asarray(w1), np.asarray(b1)
    w2, b2 = np.asarray(w2), np.asarray(b2)
    masks = np.asarray(masks)
    steps = masks.shape[0]
    apply_b2 = bool(np.any(b2 != 0))
    key = ("prog", steps, apply_b2)
    if key not in _cache:
        _cache[key] = _build_program(steps, apply_b2)
    nc = _cache[key]

    from concourse.bass_utils import run_bass_kernel_spmd

    tapw, w2b, b2r, b1v = _prep_weights(w1, b1, w2, b2)
    s_bufs = _prep_state(state)
    m_bufs = _prep_masks(masks)

    in_maps = []
    for core in range(N_CORES):
        in_maps.append({
            "s0": s_bufs[core],
            "masks": m_bufs[core],
            "tapw": tapw,
            "w2b": w2b,
            "b2r": b2r,
            "b1v": b1v,
        })

    import os
    trace = bool(os.environ.get("NCA_TRACE"))
    kw = {}
    if trace:
        kw["trace"] = True
        if os.environ.get("NCA_TRACE_DIR"):
            kw["tmpdir"] = os.environ["NCA_TRACE_DIR"]
    res = run_bass_kernel_spmd(nc, in_maps, list(range(N_CORES)), **kw)
    global LAST_EXEC_NS
    LAST_EXEC_NS = res.exec_time_ns

    out = np.zeros((B, C, H, W), np.float32)
    for core in range(N_CORES):
        o = res.results[core]["out"]  # [128, SR*W]
        b = core // 2
        top = (core % 2) == 0
        r0 = 0 if top else H - SH
        own0 = 0 if top else H // 2
        for g in range(4):
            rows = o[32 * g:32 * g + 24].reshape(C, SR, W)
            g0 = r0 + g * SR
            lo = max(g0, own0)
            hi = min(g0 + SR, own0 + H // 2)
            if lo < hi:
                out[b, :, lo:hi, :] = rows[:, lo - g0:hi - g0, :]
    return out


# revision 27
# speedup vs baseline: 62.5342x; 62.5342x over previous
"""BasicNCA (neural cellular automaton) Trainium2 kernel, 8-core SPMD.

Reference computation (per step, 32 steps):
  p  = depthwise3x3(s, [identity, sobel_x, sobel_y])   # (B, 3C, H, W)
  h  = relu(w1 @ p + b1)                               # (B, 64, H, W)
  d  = w2 @ h + b2                                     # (B, C, H, W)
  s += d * (mask < 0.5)

Implementation notes:
- The perception conv + first 1x1 conv fuse into one effective 3x3 conv with
  weights Weff[o, c, dy, dx]; computed on the PE as 9 shifted "tap" matmuls
  accumulating in PSUM (fp32r operands, full rate at free dim >= 256).
- Sharding: core i handles batch i//2, H-half i%2, with a 32-row taper of
  redundant compute instead of per-step halo exchange between cores
  (validity shrinks 1 row/step; 32 steps consume exactly the margin).
- A core's 96-row slab splits into 4 sub-slabs of 24 rows on the 4 SBUF
  partition quadrants (channels 0-23 of each), so DVE/ACT elementwise work
  runs ~120 lanes wide.
- This walrus build rejects column tile positions != 0 and crashes on PSUM
  accumulation chains that change tile_position mid-chain. So: taps process
  slab PAIRS with K=56 block lhsT (both slabs' h stacked in the output's
  partition halves, one tile_position per chain); the second 1x1 conv runs
  in full 128x128 mode (K=128 over both slabs' h, M=120 with per-slab
  24-column blocks so deltas land on the right partition quadrants).
- Masks are pre-replicated across channel lanes on the host and streamed per
  step (DMA is otherwise idle).
"""

import sys

sys.path.insert(0, "/opt/trn_rl_repo")

import numpy as np

import concourse.bass as bass
import concourse.bacc as bacc
import concourse.tile as tile
import concourse.mybir as mybir

dt = mybir.dt

B, C, H, W = 4, 24, 128, 128
HID = 64
FIRE_RATE = 0.5
N_CORES = 8

SH = 96            # slab rows per core (64 own + 32 taper)
SR = 24            # rows per sub-slab (one partition quadrant)
FW = W + 2         # padded row width (130)
FR = SR + 2        # frame rows per sub-slab (26)
FRAME_OFF = 4      # leading guard elems so tap offset -1 stays in-bounds
FRAME = FR * FW    # 3380
S_FREE = FRAME_OFF + FRAME + 4
COMP = SR * FW     # 3120 compact free size (real rows 0..23)
NCH = 390          # chunk = 3 rows
NCHUNK = COMP // NCH  # 8

LAST_EXEC_NS = None
_cache = {}


def _taps():
    # correlation taps: out(y, x) = sum_{dy,dx} in(y+dy-1, x+dx-1) * k[dy, dx]
    return [(dy, dx) for dy in range(3) for dx in range(3)]


def _build_program(steps, apply_b2, repeats=1):
    nc = bacc.Bacc("TRN2", target_bir_lowering=False, debug=False,
                   num_devices=N_CORES)

    s_d = nc.dram_tensor("s0", [128, S_FREE], dt.float32r, kind="ExternalInput")
    m_d = nc.dram_tensor("masks", [steps, 128, COMP], dt.float32,
                         kind="ExternalInput")
    tapw_d = nc.dram_tensor("tapw", [128, 9 * 128], dt.float32r,
                            kind="ExternalInput")
    w2b_d = nc.dram_tensor("w2b", [128, 2 * 120], dt.float32r,
                           kind="ExternalInput")
    b2r_d = nc.dram_tensor("b2r", [128, 1], dt.float32, kind="ExternalInput")
    b1_d = nc.dram_tensor("b1v", [128, 1], dt.float32, kind="ExternalInput")
    out_d = nc.dram_tensor("out", [128, SR * W], dt.float32,
                           kind="ExternalOutput")

    with tile.TileContext(nc) as tc:
        with tc.tile_pool(name="persist", bufs=1) as pp, \
             tc.tile_pool(name="mpool", bufs=2) as mpool, \
             tc.tile_pool(name="hsb", bufs=3) as hsbp, \
             tc.tile_pool(name="upool", bufs=2) as upool, \
             tc.tile_pool(name="hps", bufs=3, space="PSUM") as hps_pool, \
             tc.tile_pool(name="dps", bufs=1, space="PSUM") as dps_pool:

            s_sb = pp.tile([128, S_FREE], dt.float32r)
            tapw = pp.tile([128, 9 * 128], dt.float32r)
            w2b = pp.tile([128, 2 * 120], dt.float32r)
            b2r = pp.tile([128, 1], dt.float32)
            b1v = pp.tile([128, 1], dt.float32)

            nc.sync.dma_start(s_sb[:], s_d[:])
            nc.sync.dma_start(tapw[:], tapw_d[:])
            nc.sync.dma_start(w2b[:], w2b_d[:])
            nc.sync.dma_start(b2r[:], b2r_d[:])
            nc.sync.dma_start(b1v[:], b1_d[:])

            taps = _taps()

            for t in range(steps * repeats):
                t = t % steps
                m_sb = mpool.tile([128, COMP], dt.float32, tag="m")
                nc.sync.dma_start(m_sb[:], m_d[t])

                u_sb = upool.tile([128, COMP], dt.float32, tag="u")

                # zigzag chunk order: consecutive steps meet at the same
                # edge, shortening the serial step-boundary chain
                korder = (range(NCHUNK // 2) if t % 2 == 0
                          else range(NCHUNK // 2 - 1, -1, -1))
                for k in korder:                    # chunk pairs
                    hsb_list = []
                    for p in range(2):              # slab pairs {0,1} / {2,3}
                        base = 64 * p
                        hps = hps_pool.tile([128, 1024], dt.float32,
                                            tag="hps")
                        for cc in range(2):
                            c = 2 * k + cc
                            for ti, (dy, dx) in enumerate(taps):
                                off = (FRAME_OFF + (3 * c + dy) * FW
                                       + dx - 1)
                                nc.tensor.matmul(
                                    hps[:, 512 * cc:512 * cc + NCH],
                                    tapw[base:base + 56,
                                         128 * ti:128 * ti + 128],
                                    s_sb[base:base + 56, off:off + NCH],
                                    start=(ti == 0), stop=(ti == 8),
                                    tile_position=(base, 0),
                                )
                        # relu + b1 for both chunks of this slab pair
                        hsb = hsbp.tile([128, 2 * NCH], dt.float32r,
                                        tag=f"hsb{p}")
                        hsb_list.append(hsb)
                        nc.scalar.activation(
                            hsb[:].rearrange("p (b x) -> p b x", x=NCH),
                            hps[:].rearrange("p (b x) -> p b x", b=2)[:, :, 0:NCH],
                            mybir.ActivationFunctionType.Relu,
                            bias=b1v[:, 0:1],
                        )

                    dps = dps_pool.tile([128, 1024], dt.float32, tag="dps")
                    for cc in range(2):
                        for p in range(2):
                            nc.tensor.matmul(
                                dps[0:120, 512 * cc:512 * cc + NCH],
                                w2b[:, 120 * p:120 * p + 120],
                                hsb_list[p][:, NCH * cc:NCH * cc + NCH],
                                start=(p == 0), stop=(p == 1),
                            )
                    if apply_b2:
                        # delta += b2 (per-partition scalar), in psum
                        nc.vector.tensor_scalar_add(
                            dps[0:120].rearrange(
                                "p (b x) -> p b x", b=2)[:, :, 0:NCH],
                            dps[0:120].rearrange(
                                "p (b x) -> p b x", b=2)[:, :, 0:NCH],
                            b2r[0:120, 0:1],
                        )

                    # u = (m < 0.5) * delta for this chunk pair
                    nc.vector.scalar_tensor_tensor(
                        u_sb[0:120, 780 * k:780 * k + 780].rearrange(
                            "p (b x) -> p b x", x=NCH),
                        m_sb[0:120, 780 * k:780 * k + 780].rearrange(
                            "p (b x) -> p b x", x=NCH),
                        FIRE_RATE,
                        dps[0:120].rearrange("p (b x) -> p b x", b=2)[:, :, 0:NCH],
                        mybir.AluOpType.is_lt,
                        mybir.AluOpType.mult,
                    )

                # s += u, split per chunk-pair so it pipelines with later
                # chunks' taps (the tap reads of neighboring rows gate each
                # piece via Tile's range tracking)
                for k in korder:
                    a = FRAME_OFF + FW + 780 * k
                    nc.vector.tensor_add(
                        s_sb[0:120, a:a + 780],
                        s_sb[0:120, a:a + 780],
                        u_sb[0:120, 780 * k:780 * k + 780],
                    )

                # intra-core halo refresh between sub-slabs
                if True:
                    for g in range(3):
                        nc.sync.dma_start(
                            s_sb[32 * g:32 * g + 24,
                                 FRAME_OFF + 25 * FW:FRAME_OFF + 25 * FW + FW],
                            s_sb[32 * (g + 1):32 * (g + 1) + 24,
                                 FRAME_OFF + FW:FRAME_OFF + FW + FW],
                        )
                        nc.sync.dma_start(
                            s_sb[32 * (g + 1):32 * (g + 1) + 24,
                                 FRAME_OFF:FRAME_OFF + FW],
                            s_sb[32 * g:32 * g + 24,
                                 FRAME_OFF + 24 * FW:FRAME_OFF + 24 * FW + FW],
                        )

            # write back real pixels (frame rows 1..24, cols 1..128)
            a0 = FRAME_OFF + FW + 1
            nc.sync.dma_start(
                out_d[:].rearrange("p (r x) -> p r x", x=W),
                s_sb[:, a0:a0 + SR * FW].rearrange(
                    "p (r x) -> p r x", x=FW)[:, :, 0:W].bitcast(dt.float32),
            )

    nc.compile()
    return nc


def _prep_weights(w1, b1, w2, b2):
    sx = np.array([[-1, 0, 1], [-2, 0, 2], [-1, 0, 1]], np.float32) / 8.0
    sy = sx.T.copy()
    ident = np.zeros((3, 3), np.float32)
    ident[1, 1] = 1.0
    # Weff[o, c, dy, dx]
    weff = (np.einsum("oc,yx->ocyx", w1[:, 0::3], ident)
            + np.einsum("oc,yx->ocyx", w1[:, 1::3], sx)
            + np.einsum("oc,yx->ocyx", w1[:, 2::3], sy)).astype(np.float32)

    # pair-tap lhsT: K=56 rows (quadrants q, q+1 channels), M=128
    # rows 0-23 -> h of even slab at out partitions 0-63,
    # rows 32-55 -> h of odd slab at out partitions 64-127.
    tapw = np.zeros((128, 9 * 128), np.float32)
    for ti, (dy, dx) in enumerate(_taps()):
        wt = weff[:, :, dy, dx].T          # [24, 64]
        for p in range(2):
            base = 64 * p
            tapw[base:base + 24, 128 * ti:128 * ti + 64] = wt
            tapw[base + 32:base + 56, 128 * ti + 64:128 * ti + 128] = wt

    # layer2 lhsT per pair: K=128 (both h halves), M=120 with 24-col blocks
    # placing each slab's delta on its partition quadrant.
    w2b = np.zeros((128, 2 * 120), np.float32)
    for p in range(2):
        ge, go = 2 * p, 2 * p + 1
        w2b[0:64, 120 * p + 32 * ge:120 * p + 32 * ge + 24] = w2.T
        w2b[64:128, 120 * p + 32 * go:120 * p + 32 * go + 24] = w2.T

    b2r = np.zeros((128, 1), np.float32)
    b1v = np.zeros((128, 1), np.float32)
    for g in range(4):
        b2r[32 * g:32 * g + 24, 0] = b2
    b1v[0:64, 0] = b1
    b1v[64:128, 0] = b1
    return tapw, w2b, b2r, b1v


def _prep_state(state):
    """state (B, C, H, W) -> per-core [128, S_FREE] framed slabs."""
    bufs = []
    for core in range(N_CORES):
        b = core // 2
        top = (core % 2) == 0
        r0 = 0 if top else H - SH
        buf = np.zeros((128, S_FREE), np.float32)
        for ch in range(C):
            full = np.zeros((SH + 2, FW), np.float32)
            full[1:SH + 1, 1:W + 1] = state[b, ch, r0:r0 + SH, :]
            if r0 > 0:
                full[0, 1:W + 1] = state[b, ch, r0 - 1, :]
            if r0 + SH < H:
                full[SH + 1, 1:W + 1] = state[b, ch, r0 + SH, :]
            for g in range(4):
                fr = full[g * SR:g * SR + FR, :]
                buf[32 * g + ch, FRAME_OFF:FRAME_OFF + FRAME] = fr.reshape(-1)
        bufs.append(buf)
    return bufs


def _prep_masks(masks):
    """masks (S, B, 1, H, W) -> per-core [S, 128, COMP] fire-padded."""
    S = masks.shape[0]
    bufs = []
    for core in range(N_CORES):
        b = core // 2
        top = (core % 2) == 0
        r0 = 0 if top else H - SH
        mb = np.ones((S, 128, COMP), np.float32)
        mrows = np.ones((S, SH, FW), np.float32)
        mrows[:, :, 1:W + 1] = masks[:, b, 0, r0:r0 + SH, :]
        for g in range(4):
            seg = mrows[:, g * SR:(g + 1) * SR, :].reshape(S, COMP)
            mb[:, 32 * g:32 * g + C, :] = seg[:, None, :]
        bufs.append(mb)
    return bufs


def kernel(state, w1, b1, w2, b2, masks):
    state = np.asarray(state)
    w1, b1 = np.asarray(w1), np.asarray(b1)
    w2, b2 = np.asarray(w2), np.asarray(b2)
    masks = np.asarray(masks)
    import os as _os
    steps = masks.shape[0]
    apply_b2 = bool(np.any(b2 != 0))
    repeats = int(_os.environ.get("NCA_REPEAT", "1"))
    key = ("prog", steps, apply_b2, repeats)
    if key not in _cache:
        _cache[key] = _build_program(steps, apply_b2, repeats)
    nc = _cache[key]

    from concourse.bass_utils import run_bass_kernel_spmd

    tapw, w2b, b2r, b1v = _prep_weights(w1, b1, w2, b2)
    s_bufs = _prep_state(state)
    m_bufs = _prep_masks(masks)

    in_maps = []
    for core in range(N_CORES):
        in_maps.append({
            "s0": s_bufs[core],
            "masks": m_bufs[core],
            "tapw": tapw,
            "w2b": w2b,
            "b2r": b2r,
            "b1v": b1v,
        })

    import os
    trace = bool(os.environ.get("NCA_TRACE"))
    kw = {}
    if trace:
        kw["trace"] = True
        if os.environ.get("NCA_TRACE_DIR"):
            kw["tmpdir"] = os.environ["NCA_TRACE_DIR"]
    res = run_bass_kernel_spmd(nc, in_maps, list(range(N_CORES)), **kw)
    global LAST_EXEC_NS
    LAST_EXEC_NS = res.exec_time_ns

    out = np.zeros((B, C, H, W), np.float32)
    for core in range(N_CORES):
        o = res.results[core]["out"]  # [128, SR*W]
        b = core // 2
        top = (core % 2) == 0
        r0 = 0 if top else H - SH
        own0 = 0 if top else H // 2
        for g in range(4):
            rows = o[32 * g:32 * g + 24].reshape(C, SR, W)
            g0 = r0 + g * SR
            lo = max(g0, own0)
            hi = min(g0 + SR, own0 + H // 2)
            if lo < hi:
                out[b, :, lo:hi, :] = rows[:, lo - g0:hi - g0, :]
    return out
